# revision 1
# baseline (speedup 1.0000x reference)
"""Causal self-attention (B=4, T=2048, C=1024, H=16) on 8 TRN2 NeuronCores.

Sharding: core c handles batch b = c//2 and head-group g = c%2 (8 of 16
heads).  Each core computes its QKV projection slice, causal attention for
its 8 heads, and a row-parallel partial of the output projection, returning
out_t[c] = (w_proj[:, cols_g] @ Y_g[b].T) in [C, T] layout.  Host gather:
out[b] = (out_t[2b] + out_t[2b+1]).T + b_proj.

On-chip dataflow (all matmul operands float32r = full-rate TF32-like):
  x [T, C] --PE-transpose--> xT chunks [C, 512]
  qT = (w_q xT), kT = (w_k xT)        [C_local, T], 1/sqrt(hd) folded into w_q
  v  = (xT_blk^T w_v^T)               [T, C_local], ones column per head
  per head: sT[kt] = kT_blk^T qT  -> exp (ACT) -> diag tri-mask (DVE)
            yT[65, q] += v_blk^T p  (row 64 = softmax denominator)
            yT_norm = yT[0:64] * broadcast(1/denom)  (PE bcast + DVE mul)
  out_t = w_proj_t^T yT_norm          [C, T]

Biases: setup_inputs() defines b_attn = b_proj = 0; b_proj is still added
host-side, b_attn is asserted zero.
"""

import math

import numpy as np

import concourse.bacc as bacc
import concourse.mybir as mybir
import concourse.tile as tile
from concourse.bass_utils import run_bass_kernel_spmd
from concourse.masks import make_identity, make_upper_triangular

N_CORES = 8
B, T, C = 4, 2048, 1024
H, HD = 16, 64
HL = H // 2                 # local heads per core (8)
CL = HL * HD                # local qkv width (512)
P = 128
NCT = C // P                # 8 c-tiles
NTT = T // P                # 16 token tiles
NPT = CL // P               # 4 partition tiles of local q/k/v/y rows
VW = HD + 1                 # v columns per head incl. ones column (65)
F32R = mybir.dt.float32r
F32 = mybir.dt.float32
AF = mybir.ActivationFunctionType

_CACHED_NC = None


def build_nc():
    nc = bacc.Bacc("TRN2", target_bir_lowering=False, num_devices=N_CORES)
    x = nc.dram_tensor("x", [T, C], F32R, kind="ExternalInput")
    w_qkv_t = nc.dram_tensor("w_qkv_t", [C, 3 * CL], F32R, kind="ExternalInput")
    w_proj_t = nc.dram_tensor("w_proj_t", [CL, C], F32R, kind="ExternalInput")
    out_t = nc.dram_tensor("out_t", [C, T], F32, kind="ExternalOutput")

    with tile.TileContext(nc) as tc:
        with (
            tc.tile_pool(name="const", bufs=1) as constp,
            tc.tile_pool(name="qkv", bufs=1) as qkv,
        ):
            consts = constp.tile([P, 2 * P + 1], F32R, tag="consts")
            cscratch = constp.tile([P, 2 * P + 1], F32, tag="cscratch")
            make_identity(nc, cscratch[:, 0:P])
            make_upper_triangular(nc, cscratch[:, P : 2 * P], val=1.0,
                                  diag=True)
            nc.gpsimd.memset(cscratch[:, 2 * P : 2 * P + 1], 1.0)
            nc.vector.tensor_copy(consts[:], cscratch[:])
            ident = consts[:, 0:P]
            tri = consts[:, P : 2 * P]
            ones_col = consts[:, 2 * P : 2 * P + 1]

            qT = [qkv.tile([P, T], F32R, tag=f"qT{i}", name=f"qT{i}")
                  for i in range(NPT)]
            kT = [qkv.tile([P, T], F32R, tag=f"kT{i}", name=f"kT{i}")
                  for i in range(NPT)]
            # all 16 token-tiles of v packed in one tile: col = tt*520+h*65+e
            vt = qkv.tile([P, NTT * HL * VW], F32R, tag="vt", name="vt")

            # ---- phase A/B: x transpose + QKV projections, 512-token chunks
            with (
                tc.tile_pool(name="wq", bufs=1) as wqp,
                tc.tile_pool(name="xTc", bufs=2) as xtp,
                tc.tile_pool(name="xstage", bufs=2) as xstage,
                tc.tile_pool(name="tpsum", bufs=2, space="PSUM") as tpsum,
                tc.tile_pool(name="psB", bufs=4, space="PSUM") as psB,
            ):
                w_sb = []
                for kt in range(NCT):
                    w_kt = wqp.tile([P, 3 * CL], F32R, tag=f"w{kt}",
                                    name=f"w{kt}")
                    w_sb.append(w_kt)
                for part in range(3):  # Q cols first: m-loop starts sooner
                    for kt in range(NCT):
                        nc.gpsimd.dma_start(
                            w_sb[kt][:, part * CL : (part + 1) * CL],
                            w_qkv_t.ap()[
                                kt * P : (kt + 1) * P,
                                part * CL : (part + 1) * CL,
                            ],
                        )

                for tch in range(4):  # 512-token chunk
                    xT = [xtp.tile([P, 512], F32R, tag=f"xTc{ct}",
                                   name=f"xTc{ct}") for ct in range(NCT)]
                    for t4 in range(4):
                        tt = tch * 4 + t4
                        xs = xstage.tile([P, C], F32R, tag="xs", name="xs")
                        nc.sync.dma_start(
                            xs[:], x.ap()[tt * P : (tt + 1) * P, :]
                        )
                        for ct in range(NCT):
                            tp = tpsum.tile([P, P], F32R, tag="tp", name="tp")
                            nc.tensor.transpose(
                                tp[:], xs[:, ct * P : (ct + 1) * P], ident
                            )
                            nc.vector.tensor_copy(
                                xT[ct][:, t4 * P : (t4 + 1) * P], tp[:]
                            )
                    # Q^T/K^T rows m*128 for this token chunk
                    for m in range(2 * NPT):
                        ps = psB.tile([P, 512], F32, tag="psB", name="psB")
                        for kt in range(NCT):
                            nc.tensor.matmul(
                                ps[:],
                                w_sb[kt][:, m * P : (m + 1) * P],
                                xT[kt][:],
                                start=(kt == 0),
                                stop=(kt == NCT - 1),
                            )
                        dst = qT[m] if m < NPT else kT[m - NPT]
                        nc.vector.tensor_copy(
                            dst[:, tch * 512 : (tch + 1) * 512], ps[:]
                        )
                    # V for the 4 token tiles of this chunk
                    for t4 in range(4):
                        tt = tch * 4 + t4
                        ps = psB.tile([P, CL], F32, tag="psB", name="psB")
                        for kt in range(NCT):
                            nc.tensor.matmul(
                                ps[:],
                                xT[kt][:, t4 * P : (t4 + 1) * P],
                                w_sb[kt][:, 2 * CL : 3 * CL],
                                start=(kt == 0),
                                stop=(kt == NCT - 1),
                            )
                        v3 = vt[:, tt * HL * VW : (tt + 1) * HL * VW].rearrange(
                            "p (h e) -> p h e", e=VW
                        )
                        v3_f32 = vt[:].bitcast(F32)[
                            :, tt * HL * VW : (tt + 1) * HL * VW
                        ].rearrange("p (h e) -> p h e", e=VW)
                        nc.gpsimd.memset(v3_f32[:, :, HD : HD + 1], 1.0)
                        nc.vector.tensor_copy(
                            v3[:, :, 0:HD],
                            ps[:].rearrange("p (h e) -> p h e", e=HD),
                        )

            # ---- phase C: attention (qh outer) + interleaved phase D ----
            _yTp_cm = tc.tile_pool(name="yTp", bufs=1)
            yTp = _yTp_cm.__enter__()
            yT = [yTp.tile([P, T], F32R, tag=f"yT{i}", name=f"yT{i}")
                  for i in range(NPT)]
            with (
                tc.tile_pool(name="psS", bufs=2, space="PSUM") as psS,
                tc.tile_pool(name="psY", bufs=4, space="PSUM") as psY,
                tc.tile_pool(name="pP", bufs=5) as pPp,
                tc.tile_pool(name="rc", bufs=4) as rcp,
                tc.tile_pool(name="bcs", bufs=2) as bcsp,
                tc.tile_pool(name="yraw", bufs=4) as yrawp,
                tc.tile_pool(name="wp", bufs=1) as wpp,
                tc.tile_pool(name="ost", bufs=3) as ostp,
            ):
                wts_all = []
                for m in range(NCT):
                    wts = wpp.tile([P, NPT * P], F32R, tag=f"wp{m}",
                                   name=f"wts{m}")
                    for kt in range(NPT):
                        nc.gpsimd.dma_start(
                            wts[:, kt * P : (kt + 1) * P],
                            w_proj_t.ap()[
                                kt * P : (kt + 1) * P, m * P : (m + 1) * P
                            ],
                        )
                    wts_all.append(wts)

                def emit_d(qc_d):
                    # output projection for a finished q-quarter
                    qd0 = qc_d * 512
                    for m in range(NCT):
                        ps = psS.tile([P, 512], F32, tag="psS", name="psD")
                        for kt in range(NPT):
                            nc.tensor.matmul(
                                ps[:],
                                wts_all[m][:, kt * P : (kt + 1) * P],
                                yT[kt][:, qd0 : qd0 + 512],
                                start=(kt == 0),
                                stop=(kt == NPT - 1),
                            )
                        ob = ostp.tile([P, 512], F32, tag="ost", name="ob")
                        nc.vector.tensor_copy(ob[:], ps[:])
                        nc.sync.dma_start(
                            out_t.ap()[m * P : (m + 1) * P, qd0 : qd0 + 512],
                            ob[:],
                        )

                for qc in range(4):  # q-quarter [qc*512, +512)
                    q0 = qc * 512
                    n_kt = (q0 + 512) // P
                    for duo in range(2):  # pairs (2*duo, 2*duo+1) interleaved
                        hps = (2 * duo, 2 * duo + 1)
                        y_ps = {
                            (p, hs): psY.tile([VW, 512], F32, tag="psY",
                                              name="y_ps")
                            for p in range(2) for hs in range(2)
                        }

                        def s_exp_stage(hp, kt):
                            j0 = kt * P
                            o = max(0, j0 - q0)
                            s_pair = psS.tile([P, 1024], F32, tag="psS",
                                              name="s_pair")
                            for hs in range(2):
                                r0 = hs * HD
                                nc.tensor.matmul(
                                    s_pair[:, 512 * hs + o : 512 * (hs + 1)],
                                    kT[hp][r0 : r0 + HD, j0 : j0 + P],
                                    qT[hp][r0 : r0 + HD, q0 + o : q0 + 512],
                                    start=True,
                                    stop=True,
                                    tile_position=(r0, 0),
                                )
                            pt = pPp.tile([P, 1024], F32R, tag="pP",
                                          name="pt")
                            s3 = s_pair[:].rearrange("p (h e) -> p h e", e=512)
                            p3 = pt[:].rearrange("p (h e) -> p h e", e=512)
                            nc.scalar.activation(
                                p3[:, :, o:512], s3[:, :, o:512], AF.Exp
                            )
                            if j0 >= q0:
                                tri3 = tri[:, None, :].broadcast_to([P, 2, P])
                                nc.vector.tensor_mul(
                                    p3[:, :, o : o + P], p3[:, :, o : o + P],
                                    tri3,
                                )
                            return pt

                        def y_stage(pidx, hp, kt, pt, first, final):
                            j0 = kt * P
                            o = max(0, j0 - q0)
                            for hs in range(2):
                                h = 2 * hp + hs
                                nc.tensor.matmul(
                                    y_ps[(pidx, hs)][:, o:512],
                                    vt[:, kt * HL * VW + h * VW
                                       : (kt * HL * VW + h * VW) + VW],
                                    pt[:, 512 * hs + o : 512 * (hs + 1)],
                                    start=first,
                                    stop=final,
                                )

                        # two pairs interleaved, depth-1 pipeline: while ACT
                        # exps one pair, PE runs the other pair's S and both
                        # pairs' y from the previous kt. Diagonal kts first.
                        kts = list(range(qc * 4, n_kt)) + list(range(qc * 4))
                        pts_prev = None
                        for i in range(n_kt + 1):
                            pts_new = None
                            if i < n_kt:
                                pts_new = [
                                    s_exp_stage(hps[p], kts[i])
                                    for p in range(2)
                                ]
                            if pts_prev is not None:
                                for p in range(2):
                                    y_stage(p, hps[p], kts[i - 1],
                                            pts_prev[p], first=(i == 1),
                                            final=(i == n_kt))
                            pts_prev = pts_new

                        # normalization, grouped by op so ACT does
                        # [4x Ln][4x Exp] -- each Ln<->Exp transition costs a
                        # ~1.3us ACT_TABLE_LOAD that would otherwise break
                        # the softmax exp stream 64 times
                        y_raws = []
                        for p in range(2):
                            for hs in range(2):
                                y_raw = yrawp.tile([VW, 512], F32, tag="yraw",
                                                   name="y_raw")
                                nc.vector.tensor_copy(
                                    y_raw[:], y_ps[(p, hs)][:]
                                )
                                y_raws.append((p, hs, y_raw))
                        ln_ds = []
                        for p, hs, y_raw in y_raws:
                            ln_d = rcp.tile([1, 512], F32, tag="lnd",
                                            name="ln_d")
                            nc.scalar.activation(
                                ln_d[:], y_raw[HD : HD + 1, :], AF.Ln
                            )
                            ln_ds.append(ln_d)
                        for (p, hs, y_raw), ln_d in zip(y_raws, ln_ds):
                            rc = rcp.tile([1, 512], F32, tag="rc", name="rc")
                            nc.scalar.activation(
                                rc[:], ln_d[:], AF.Exp, scale=-1.0
                            )
                            bc_sb = bcsp.tile([P, 512], F32, tag="bcs",
                                              name="bc_sb")
                            nc.gpsimd.partition_broadcast(bc_sb[:], rc[:])
                            nc.vector.tensor_mul(
                                yT[hps[p]][hs * HD : (hs + 1) * HD,
                                           q0 : q0 + 512],
                                y_raw[0:HD, :],
                                bc_sb[0:HD, :],
                            )
                    emit_d(qc)
            _yTp_cm.__exit__(None, None, None)
    nc.compile()
    return nc


def make_in_maps(x, w_attn, b_attn, w_proj):
    scale = 1.0 / math.sqrt(HD)
    in_maps = []
    for core in range(N_CORES):
        b = core // 2
        g = core % 2
        h0 = g * HL
        rows = np.arange(h0 * HD, (h0 + HL) * HD)
        w_q = w_attn[rows, :] * scale           # fold 1/sqrt(hd) into Q
        w_k = w_attn[C + rows, :]
        w_v = w_attn[2 * C + rows, :]
        w_qkv_t = np.ascontiguousarray(
            np.concatenate([w_q, w_k, w_v], axis=0).T
        ).astype(np.float32)
        w_proj_t = np.ascontiguousarray(w_proj[:, rows].T).astype(np.float32)
        in_maps.append(
            {
                "x": np.ascontiguousarray(x[b]).astype(np.float32),
                "w_qkv_t": w_qkv_t,
                "w_proj_t": w_proj_t,
            }
        )
    return in_maps


def _run(in_maps, trace=False, **kw):
    global _CACHED_NC
    if _CACHED_NC is None:
        _CACHED_NC = build_nc()
    return run_bass_kernel_spmd(
        _CACHED_NC, in_maps, core_ids=list(range(N_CORES)), trace=trace, **kw
    )


def kernel(x, w_attn, b_attn, w_proj, b_proj):
    x = np.asarray(x, dtype=np.float32)
    w_attn = np.asarray(w_attn, dtype=np.float32)
    b_attn = np.asarray(b_attn, dtype=np.float32)
    w_proj = np.asarray(w_proj, dtype=np.float32)
    b_proj = np.asarray(b_proj, dtype=np.float32)
    assert not np.any(b_attn), "kernel assumes b_attn == 0 (as in setup_inputs)"
    res = _run(make_in_maps(x, w_attn, b_attn, w_proj))
    out = np.empty((B, T, C), dtype=np.float32)
    for b in range(B):
        p0 = res.results[2 * b]["out_t"]
        p1 = res.results[2 * b + 1]["out_t"]
        out[b] = (p0 + p1).T + b_proj
    return out



# revision 7
# speedup vs baseline: 1.0787x; 1.0787x over previous
"""Causal self-attention (B=4, T=2048, C=1024, H=16) on 8 TRN2 NeuronCores.

Sharding: core c handles batch b = c//2 and head-group g = c%2 (8 of 16
heads).  Each core computes its QKV projection slice, causal attention for
its 8 heads, and a row-parallel partial of the output projection, returning
out_t[c] = (w_proj[:, cols_g] @ Y_g[b].T) in [C, T] layout.  Host gather:
out[b] = (out_t[2b] + out_t[2b+1]).T + b_proj.

On-chip dataflow (matmul operands bf16, PSUM accumulation f32):
  x [T, C] --PE-transpose--> xT chunks [C, 512]
  qT = (w_q xT), kT = (w_k xT)        [C_local, T], 1/sqrt(hd) folded into w_q
  v  = (xT_blk^T w_v^T)               [T, C_local], ones column per head
  per head-pair: sT[kt] = kT_blk^T qT (row-tiled 64x128, both heads overlap)
            exp (ACT, one 2D instr for full tiles) -> pt bf16
            diag tiles: tri-mask (DVE) -- full tiles issued first so the
            'start' matmul covers the whole PSUM bank
            yT[65, q] += v_blk^T p  (row 0 = softmax denominator)
  duo end:  rc = 1/denom (DVE reciprocal), GpSimd partition-broadcast,
            yT_norm = y * bc -- muls deferred into the next duo's stages
  out_t = w_proj_t^T yT_norm -- matmul groups deferred/interleaved into the
            next quarter's stages so the PE queue never stalls on the norm
            chain (stalls cool the PE clock-gate to 1.2 GHz).

Biases: setup_inputs() defines b_attn = b_proj = 0; b_proj is still added
host-side, b_attn is asserted zero.
"""

import math

import ml_dtypes
import numpy as np

import concourse.bacc as bacc
import concourse.mybir as mybir
import concourse.tile as tile
from concourse.bass_utils import run_bass_kernel_spmd
from concourse.masks import make_identity, make_upper_triangular

N_CORES = 8
B, T, C = 4, 2048, 1024
H, HD = 16, 64
HL = H // 2                 # local heads per core (8)
CL = HL * HD                # local qkv width (512)
P = 128
NCT = C // P                # 8 c-tiles
NTT = T // P                # 16 token tiles
NPT = CL // P               # 4 partition tiles of local q/k/v/y rows
VW = HD + 1                 # v columns per head incl. leading ones column (65)
BF16 = mybir.dt.bfloat16
F32 = mybir.dt.float32
AF = mybir.ActivationFunctionType
NPBF16 = ml_dtypes.bfloat16

_CACHED_NC = None


def build_nc():
    nc = bacc.Bacc("TRN2", target_bir_lowering=False, num_devices=N_CORES)
    x = nc.dram_tensor("x", [T, C], BF16, kind="ExternalInput")
    w_qkv_t = nc.dram_tensor("w_qkv_t", [C, 3 * CL], BF16, kind="ExternalInput")
    w_proj_t = nc.dram_tensor("w_proj_t", [CL, C], BF16, kind="ExternalInput")
    out_t = nc.dram_tensor("out_t", [C, T], F32, kind="ExternalOutput")

    with tile.TileContext(nc) as tc:
        with (
            tc.tile_pool(name="const", bufs=1) as constp,
            tc.tile_pool(name="qkv", bufs=1) as qkv,
        ):
            consts = constp.tile([P, 2 * P], BF16, tag="consts")
            cscratch = constp.tile([P, 2 * P], F32, tag="cscratch")
            make_identity(nc, cscratch[:, 0:P])
            make_upper_triangular(nc, cscratch[:, P : 2 * P], val=1.0,
                                  diag=True)
            nc.vector.tensor_copy(consts[:], cscratch[:])
            ident = consts[:, 0:P]
            tri = consts[:, P : 2 * P]

            qT = [qkv.tile([P, T], BF16, tag=f"qT{i}", name=f"qT{i}")
                  for i in range(NPT)]
            kT = [qkv.tile([P, T], BF16, tag=f"kT{i}", name=f"kT{i}")
                  for i in range(NPT)]
            # all 16 token-tiles of v packed in one tile: col = tt*520+h*65+e
            # e=0 is the ones column (denominator lands on PSUM partition 0)
            vt = qkv.tile([P, NTT * HL * VW], BF16, tag="vt", name="vt")

            # ---- phase A/B: x transpose + QKV projections, 512-token chunks
            with (
                tc.tile_pool(name="wq", bufs=1) as wqp,
                tc.tile_pool(name="xTc", bufs=2) as xtp,
                tc.tile_pool(name="xstage", bufs=2) as xstage,
                tc.tile_pool(name="tpsum", bufs=2, space="PSUM") as tpsum,
                tc.tile_pool(name="psB", bufs=4, space="PSUM") as psB,
            ):
                w_sb = []
                for kt_i in range(NCT):
                    w_kt = wqp.tile([P, 3 * CL], BF16, tag=f"w{kt_i}",
                                    name=f"w{kt_i}")
                    w_sb.append(w_kt)
                for part in range(3):  # Q cols first: m-loop starts sooner
                    for kt_i in range(NCT):
                        nc.gpsimd.dma_start(
                            w_sb[kt_i][:, part * CL : (part + 1) * CL],
                            w_qkv_t.ap()[
                                kt_i * P : (kt_i + 1) * P,
                                part * CL : (part + 1) * CL,
                            ],
                        )

                for tch in range(4):  # 512-token chunk
                    xT = [xtp.tile([P, 512], BF16, tag=f"xTc{ct}",
                                   name=f"xTc{ct}") for ct in range(NCT)]
                    for t4 in range(4):
                        tt = tch * 4 + t4
                        xs = xstage.tile([P, C], BF16, tag="xs", name="xs")
                        nc.sync.dma_start(
                            xs[:], x.ap()[tt * P : (tt + 1) * P, :]
                        )
                        for ct in range(NCT):
                            tp = tpsum.tile([P, P], BF16, tag="tp", name="tp")
                            nc.tensor.transpose(
                                tp[:], xs[:, ct * P : (ct + 1) * P], ident
                            )
                            nc.vector.tensor_copy(
                                xT[ct][:, t4 * P : (t4 + 1) * P], tp[:]
                            )
                    # Q^T/K^T rows m*128 for this token chunk
                    for m in range(2 * NPT):
                        ps = psB.tile([P, 512], F32, tag="psB", name="psB")
                        for kt_i in range(NCT):
                            nc.tensor.matmul(
                                ps[:],
                                w_sb[kt_i][:, m * P : (m + 1) * P],
                                xT[kt_i][:],
                                start=(kt_i == 0),
                                stop=(kt_i == NCT - 1),
                            )
                        dst = qT[m] if m < NPT else kT[m - NPT]
                        nc.vector.tensor_copy(
                            dst[:, tch * 512 : (tch + 1) * 512], ps[:]
                        )
                    # V for the 4 token tiles of this chunk
                    for t4 in range(4):
                        tt = tch * 4 + t4
                        ps = psB.tile([P, CL], F32, tag="psB", name="psB")
                        for kt_i in range(NCT):
                            nc.tensor.matmul(
                                ps[:],
                                xT[kt_i][:, t4 * P : (t4 + 1) * P],
                                w_sb[kt_i][:, 2 * CL : 3 * CL],
                                start=(kt_i == 0),
                                stop=(kt_i == NCT - 1),
                            )
                        v3 = vt[:, tt * HL * VW : (tt + 1) * HL * VW].rearrange(
                            "p (h e) -> p h e", e=VW
                        )
                        nc.gpsimd.memset(v3[:, :, HD : HD + 1], 1.0)
                        nc.vector.tensor_copy(
                            v3[:, :, 0:HD],
                            ps[:].rearrange("p (h e) -> p h e", e=HD),
                        )

            # ---- phase C: attention; norm + phase D deferred into later
            # stage slots so the PE instruction queue never waits on them
            _yTp_cm = tc.tile_pool(name="yTp", bufs=1)
            yTp = _yTp_cm.__enter__()
            yT = [yTp.tile([P, T], BF16, tag=f"yT{i}", name=f"yT{i}")
                  for i in range(NPT)]
            with (
                tc.tile_pool(name="psS", bufs=2, space="PSUM") as psS,
                tc.tile_pool(name="psY", bufs=4, space="PSUM") as psY,
                tc.tile_pool(name="pP", bufs=4) as pPp,
                tc.tile_pool(name="rc", bufs=4) as rcp,
                tc.tile_pool(name="bcs", bufs=4) as bcsp,
                tc.tile_pool(name="wp", bufs=1) as wpp,
                tc.tile_pool(name="ost", bufs=3) as ostp,
            ):
                wts_all = []
                for m in range(NCT):
                    wts = wpp.tile([P, NPT * P], BF16, tag=f"wp{m}",
                                   name=f"wts{m}")
                    for kt_i in range(NPT):
                        nc.sync.dma_start(
                            wts[:, kt_i * P : (kt_i + 1) * P],
                            w_proj_t.ap()[
                                kt_i * P : (kt_i + 1) * P, m * P : (m + 1) * P
                            ],
                        )
                    wts_all.append(wts)

                # deferred closures (norm muls, phase-D groups), drained one
                # per stage starting at each duo's stage 1
                deferred = []

                def emit_d_group(qc_d, m):
                    # output projection for one 128-row m-tile of a finished
                    # q-quarter
                    qd0 = qc_d * 512
                    ps = psS.tile([P, 512], F32, tag="psS", name="psD")
                    for kt_i in range(NPT):
                        nc.tensor.matmul(
                            ps[:],
                            wts_all[m][:, kt_i * P : (kt_i + 1) * P],
                            yT[kt_i][:, qd0 : qd0 + 512],
                            start=(kt_i == 0),
                            stop=(kt_i == NPT - 1),
                        )
                    ob = ostp.tile([P, 512], F32, tag="ost", name="ob")
                    nc.vector.tensor_copy(ob[:], ps[:])
                    nc.sync.dma_start(
                        out_t.ap()[m * P : (m + 1) * P, qd0 : qd0 + 512],
                        ob[:],
                    )

                for qc in range(4):  # q-quarter [qc*512, +512)
                    q0 = qc * 512
                    n_kt = (q0 + 512) // P
                    for duo in range(2):  # pairs (2*duo, 2*duo+1) interleaved
                        hps = (2 * duo, 2 * duo + 1)
                        y_ps = {
                            (p, hs): psY.tile([VW, 512], F32, tag="psY",
                                              name="y_ps")
                            for p in range(2) for hs in range(2)
                        }

                        def s_exp_stage(hp, kt):
                            j0 = kt * P
                            o = max(0, j0 - q0)
                            s_pair = psS.tile([P, 1024], F32, tag="psS",
                                              name="s_pair")
                            for hs in range(2):
                                r0 = hs * HD
                                nc.tensor.matmul(
                                    s_pair[:, 512 * hs + o : 512 * (hs + 1)],
                                    kT[hp][r0 : r0 + HD, j0 : j0 + P],
                                    qT[hp][r0 : r0 + HD, q0 + o : q0 + 512],
                                    start=True,
                                    stop=True,
                                    tile_position=(r0, 0),
                                )
                            pt = pPp.tile([P, 1024], BF16, tag="pP",
                                          name="pt")
                            if o == 0:
                                # full tile: one 2D exp over both heads
                                nc.scalar.activation(pt[:], s_pair[:], AF.Exp)
                            else:
                                s3 = s_pair[:].rearrange(
                                    "p (h e) -> p h e", e=512
                                )
                                p3 = pt[:].rearrange("p (h e) -> p h e", e=512)
                                nc.scalar.activation(
                                    p3[:, :, o:512], s3[:, :, o:512], AF.Exp
                                )
                            if j0 >= q0:
                                p3 = pt[:].rearrange("p (h e) -> p h e", e=512)
                                tri3 = tri[:, None, :].broadcast_to([P, 2, P])
                                nc.vector.tensor_mul(
                                    p3[:, :, o : o + P], p3[:, :, o : o + P],
                                    tri3,
                                )
                            return pt

                        def y_stage(pidx, hp, kt, pt, first, final):
                            j0 = kt * P
                            o = max(0, j0 - q0)
                            for hs in range(2):
                                h = 2 * hp + hs
                                nc.tensor.matmul(
                                    y_ps[(pidx, hs)][:, o:512],
                                    vt[:, kt * HL * VW + h * VW
                                       : (kt * HL * VW + h * VW) + VW],
                                    pt[:, 512 * hs + o : 512 * (hs + 1)],
                                    start=first,
                                    stop=final,
                                )

                        # two pairs interleaved, depth-1 pipeline.  Full kt
                        # tiles first (their 'start' matmul clears the whole
                        # PSUM bank; diagonal tiles only touch cols o:512),
                        # diagonal tiles last -- their small exp lets ACT
                        # catch up right before the duo boundary.
                        kts = list(range(qc * 4)) + list(range(qc * 4, n_kt))
                        pts_prev = None
                        for i in range(n_kt + 1):
                            # drain deferred work first: a previous duo's
                            # norm_muls must be ISSUED before this duo's AV
                            # matmuls overwrite the recycled psY buffers,
                            # or the WAR dependency is never recorded
                            if deferred:
                                deferred.pop(0)()
                            pts_new = None
                            if i < n_kt:
                                pts_new = [
                                    s_exp_stage(hps[p], kts[i])
                                    for p in range(2)
                                ]
                            if pts_prev is not None:
                                for p in range(2):
                                    y_stage(p, hps[p], kts[i - 1],
                                            pts_prev[p], first=(i == 1),
                                            final=(i == n_kt))
                            pts_prev = pts_new

                        # normalization: reciprocal of the denominator row
                        # (PSUM partition 0) + partition-broadcast now; the
                        # multiplies go into the next duo's stage slots
                        bcs = {}
                        for p in range(2):
                            for hs in range(2):
                                rc = rcp.tile([VW, 512], F32, tag="rc",
                                              name="rc")
                                nc.vector.reciprocal(
                                    rc[HD : HD + 1, :],
                                    y_ps[(p, hs)][HD : HD + 1, :],
                                )
                                # partition_broadcast's HW ucode reads
                                # partition 0 regardless of the AP offset;
                                # DMA the row from partition 64 to 0 first
                                nc.gpsimd.dma_start(
                                    rc[0:1, :], rc[HD : HD + 1, :]
                                )
                                bc = bcsp.tile([HD, 512], F32, tag="bcs",
                                               name="bc")
                                nc.gpsimd.partition_broadcast(
                                    bc[:], rc[0:1, :]
                                )
                                bcs[(p, hs)] = bc

                        def norm_muls(y_ps=y_ps, bcs=bcs, hps=hps, q0=q0):
                            for p in range(2):
                                for hs in range(2):
                                    nc.vector.tensor_mul(
                                        yT[hps[p]][hs * HD : (hs + 1) * HD,
                                                   q0 : q0 + 512],
                                        y_ps[(p, hs)][0:HD, :],
                                        bcs[(p, hs)][:],
                                    )

                        deferred.append(norm_muls)
                    for m in range(NCT):
                        deferred.append(
                            lambda qc=qc, m=m: emit_d_group(qc, m)
                        )
                while deferred:
                    deferred.pop(0)()
            _yTp_cm.__exit__(None, None, None)
    nc.compile()
    return nc


def make_in_maps(x, w_attn, b_attn, w_proj):
    scale = 1.0 / math.sqrt(HD)
    in_maps = []
    for core in range(N_CORES):
        b = core // 2
        g = core % 2
        h0 = g * HL
        rows = np.arange(h0 * HD, (h0 + HL) * HD)
        w_q = w_attn[rows, :] * scale           # fold 1/sqrt(hd) into Q
        w_k = w_attn[C + rows, :]
        w_v = w_attn[2 * C + rows, :]
        w_qkv_t = np.ascontiguousarray(
            np.concatenate([w_q, w_k, w_v], axis=0).T
        ).astype(NPBF16)
        w_proj_t = np.ascontiguousarray(w_proj[:, rows].T).astype(NPBF16)
        in_maps.append(
            {
                "x": np.ascontiguousarray(x[b]).astype(NPBF16),
                "w_qkv_t": w_qkv_t,
                "w_proj_t": w_proj_t,
            }
        )
    return in_maps


def _run(in_maps, trace=False, **kw):
    global _CACHED_NC
    if _CACHED_NC is None:
        _CACHED_NC = build_nc()
    return run_bass_kernel_spmd(
        _CACHED_NC, in_maps, core_ids=list(range(N_CORES)), trace=trace, **kw
    )


def kernel(x, w_attn, b_attn, w_proj, b_proj):
    x = np.asarray(x, dtype=np.float32)
    w_attn = np.asarray(w_attn, dtype=np.float32)
    b_attn = np.asarray(b_attn, dtype=np.float32)
    w_proj = np.asarray(w_proj, dtype=np.float32)
    b_proj = np.asarray(b_proj, dtype=np.float32)
    assert not np.any(b_attn), "kernel assumes b_attn == 0 (as in setup_inputs)"
    res = _run(make_in_maps(x, w_attn, b_attn, w_proj))
    out = np.empty((B, T, C), dtype=np.float32)
    for b in range(B):
        p0 = res.results[2 * b]["out_t"]
        p1 = res.results[2 * b + 1]["out_t"]
        out[b] = (p0 + p1).T + b_proj
    return out


# revision 10
# speedup vs baseline: 1.3074x; 1.2120x over previous
"""Causal self-attention (B=4, T=2048, C=1024, H=16) on 8 TRN2 NeuronCores.

Sharding: core c handles batch b = c//2 and head-group g = c%2 (8 of 16
heads).  Each core computes its QKV projection slice, causal attention for
its 8 heads, and a row-parallel partial of the output projection, returning
out_t[c] = (w_proj[:, cols_g] @ Y_g[b].T) in [C, T] layout.  Host gather:
out[b] = (out_t[2b] + out_t[2b+1]).T + b_proj.

On-chip dataflow (matmul operands bf16, PSUM accumulation f32):
  x [T, C] --PE-transpose--> xT chunks [C, 512]
  qT = (w_q xT), kT = (w_k xT)        [C_local, T], 1/sqrt(hd) folded into w_q
  v  = (xT_blk^T w_v^T)               [T, C_local], ones column per head
  per head-pair: sT[kt] = kT_blk^T qT (row-tiled 64x128, both heads overlap)
            exp (ACT, one 2D instr for full tiles) -> pt bf16
            diag tiles: tri-mask (DVE) -- full tiles issued first so the
            'start' matmul covers the whole PSUM bank
            yT[65, q] += v_blk^T p  (row 0 = softmax denominator)
  duo end:  rc = 1/denom (DVE reciprocal), GpSimd partition-broadcast,
            yT_norm = y * bc -- muls deferred into the next duo's stages
  out_t = w_proj_t^T yT_norm -- matmul groups deferred/interleaved into the
            next quarter's stages so the PE queue never stalls on the norm
            chain (stalls cool the PE clock-gate to 1.2 GHz).

Biases: setup_inputs() defines b_attn = b_proj = 0; b_proj is still added
host-side, b_attn is asserted zero.
"""

import math

import ml_dtypes
import numpy as np

import concourse.bacc as bacc
import concourse.mybir as mybir
import concourse.tile as tile
from concourse.bass_utils import run_bass_kernel_spmd
from concourse.masks import make_identity, make_upper_triangular

N_CORES = 8
B, T, C = 4, 2048, 1024
H, HD = 16, 64
HL = H // 2                 # local heads per core (8)
CL = HL * HD                # local qkv width (512)
P = 128
NCT = C // P                # 8 c-tiles
NTT = T // P                # 16 token tiles
NPT = CL // P               # 4 partition tiles of local q/k/v/y rows
VW = HD + 1                 # v columns per head incl. leading ones column (65)
BF16 = mybir.dt.bfloat16
F32 = mybir.dt.float32
AF = mybir.ActivationFunctionType
NPBF16 = ml_dtypes.bfloat16

_CACHED_NC = None


def build_nc():
    nc = bacc.Bacc("TRN2", target_bir_lowering=False, num_devices=N_CORES)
    x = nc.dram_tensor("x", [T, C], BF16, kind="ExternalInput")
    w_qkv_t = nc.dram_tensor("w_qkv_t", [C, 3 * CL], BF16, kind="ExternalInput")
    w_proj_t = nc.dram_tensor("w_proj_t", [CL, C], BF16, kind="ExternalInput")
    out_t = nc.dram_tensor("out_t", [C, T], F32, kind="ExternalOutput")

    with tile.TileContext(nc) as tc:
        with (
            tc.tile_pool(name="const", bufs=1) as constp,
            tc.tile_pool(name="qkv", bufs=1) as qkv,
        ):
            consts = constp.tile([P, 2 * P], BF16, tag="consts")
            cscratch = constp.tile([P, 2 * P], F32, tag="cscratch")
            make_identity(nc, cscratch[:, 0:P])
            make_upper_triangular(nc, cscratch[:, P : 2 * P], val=1.0,
                                  diag=True)
            nc.vector.tensor_copy(consts[:], cscratch[:])
            ident = consts[:, 0:P]
            tri = consts[:, P : 2 * P]

            qT = [qkv.tile([P, T], BF16, tag=f"qT{i}", name=f"qT{i}")
                  for i in range(NPT)]
            kT = [qkv.tile([P, T], BF16, tag=f"kT{i}", name=f"kT{i}")
                  for i in range(NPT)]
            # all 16 token-tiles of v packed in one tile: col = tt*520+h*65+e
            # e=0 is the ones column (denominator lands on PSUM partition 0)
            vt = qkv.tile([P, NTT * HL * VW], BF16, tag="vt", name="vt")

            # ---- phase A/B: x transpose + QKV projections, 512-token chunks
            with (
                tc.tile_pool(name="wq", bufs=1) as wqp,
                tc.tile_pool(name="xTc", bufs=2) as xtp,
                tc.tile_pool(name="xstage", bufs=2) as xstage,
                tc.tile_pool(name="tpsum", bufs=2, space="PSUM") as tpsum,
                tc.tile_pool(name="psB", bufs=4, space="PSUM") as psB,
            ):
                w_sb = []
                for kt_i in range(NCT):
                    w_kt = wqp.tile([P, 3 * CL], BF16, tag=f"w{kt_i}",
                                    name=f"w{kt_i}")
                    w_sb.append(w_kt)
                for part in range(3):  # Q cols first: m-loop starts sooner
                    for kt_i in range(NCT):
                        nc.gpsimd.dma_start(
                            w_sb[kt_i][:, part * CL : (part + 1) * CL],
                            w_qkv_t.ap()[
                                kt_i * P : (kt_i + 1) * P,
                                part * CL : (part + 1) * CL,
                            ],
                        )

                for tch in range(4):  # 512-token chunk
                    xT = [xtp.tile([P, 512], BF16, tag=f"xTc{ct}",
                                   name=f"xTc{ct}") for ct in range(NCT)]
                    for t4 in range(4):
                        tt = tch * 4 + t4
                        xs = xstage.tile([P, C], BF16, tag="xs", name="xs")
                        nc.sync.dma_start(
                            xs[:], x.ap()[tt * P : (tt + 1) * P, :]
                        )
                        for ct in range(NCT):
                            tp = tpsum.tile([P, P], BF16, tag="tp", name="tp")
                            nc.tensor.transpose(
                                tp[:], xs[:, ct * P : (ct + 1) * P], ident
                            )
                            nc.vector.tensor_copy(
                                xT[ct][:, t4 * P : (t4 + 1) * P], tp[:]
                            )
                    # Q^T/K^T rows m*128 for this token chunk
                    for m in range(2 * NPT):
                        ps = psB.tile([P, 512], F32, tag="psB", name="psB")
                        for kt_i in range(NCT):
                            nc.tensor.matmul(
                                ps[:],
                                w_sb[kt_i][:, m * P : (m + 1) * P],
                                xT[kt_i][:],
                                start=(kt_i == 0),
                                stop=(kt_i == NCT - 1),
                            )
                        dst = qT[m] if m < NPT else kT[m - NPT]
                        nc.vector.tensor_copy(
                            dst[:, tch * 512 : (tch + 1) * 512], ps[:]
                        )
                    # V for the 4 token tiles of this chunk
                    for t4 in range(4):
                        tt = tch * 4 + t4
                        ps = psB.tile([P, CL], F32, tag="psB", name="psB")
                        for kt_i in range(NCT):
                            nc.tensor.matmul(
                                ps[:],
                                xT[kt_i][:, t4 * P : (t4 + 1) * P],
                                w_sb[kt_i][:, 2 * CL : 3 * CL],
                                start=(kt_i == 0),
                                stop=(kt_i == NCT - 1),
                            )
                        v3 = vt[:, tt * HL * VW : (tt + 1) * HL * VW].rearrange(
                            "p (h e) -> p h e", e=VW
                        )
                        nc.gpsimd.memset(v3[:, :, HD : HD + 1], 1.0)
                        nc.vector.tensor_copy(
                            v3[:, :, 0:HD],
                            ps[:].rearrange("p (h e) -> p h e", e=HD),
                        )

            # ---- phase C: attention; norm + phase D deferred into later
            # stage slots so the PE instruction queue never waits on them
            _yTp_cm = tc.tile_pool(name="yTp", bufs=1)
            yTp = _yTp_cm.__enter__()
            yT = [yTp.tile([P, T], BF16, tag=f"yT{i}", name=f"yT{i}")
                  for i in range(NPT)]
            with (
                tc.tile_pool(name="psS", bufs=2, space="PSUM") as psS,
                tc.tile_pool(name="psY", bufs=4, space="PSUM") as psY,
                tc.tile_pool(name="pP", bufs=4) as pPp,
                tc.tile_pool(name="ysb", bufs=8) as ysbp,
                tc.tile_pool(name="dent", bufs=4) as dentp,
                tc.tile_pool(name="rc", bufs=4) as rcp,
                tc.tile_pool(name="bcs", bufs=4) as bcsp,
                tc.tile_pool(name="wp", bufs=1) as wpp,
                tc.tile_pool(name="ost", bufs=3) as ostp,
            ):
                wts_all = []
                for m in range(NCT):
                    wts = wpp.tile([P, NPT * P], BF16, tag=f"wp{m}",
                                   name=f"wts{m}")
                    for kt_i in range(NPT):
                        nc.sync.dma_start(
                            wts[:, kt_i * P : (kt_i + 1) * P],
                            w_proj_t.ap()[
                                kt_i * P : (kt_i + 1) * P, m * P : (m + 1) * P
                            ],
                        )
                    wts_all.append(wts)

                # deferred closures (norm muls, phase-D groups), drained one
                # per stage starting at each duo's stage 1
                deferred = []

                def emit_d_group(qc_d, m):
                    # output projection for one 128-row m-tile of a finished
                    # q-quarter
                    qd0 = qc_d * 512
                    ps = psS.tile([P, 512], F32, tag="psS", name="psD")
                    for kt_i in range(NPT):
                        nc.tensor.matmul(
                            ps[:],
                            wts_all[m][:, kt_i * P : (kt_i + 1) * P],
                            yT[kt_i][:, qd0 : qd0 + 512],
                            start=(kt_i == 0),
                            stop=(kt_i == NPT - 1),
                        )
                    ob = ostp.tile([P, 512], F32, tag="ost", name="ob")
                    nc.vector.tensor_copy(ob[:], ps[:])
                    nc.sync.dma_start(
                        out_t.ap()[m * P : (m + 1) * P, qd0 : qd0 + 512],
                        ob[:],
                    )

                for qc in range(4):  # q-quarter [qc*512, +512)
                    q0 = qc * 512
                    n_kt = (q0 + 512) // P
                    for duo in range(2):  # pairs (2*duo, 2*duo+1) interleaved
                        hps = (2 * duo, 2 * duo + 1)
                        y_ps = {
                            (p, hs): psY.tile([VW, 512], F32, tag="psY",
                                              name="y_ps")
                            for p in range(2) for hs in range(2)
                        }

                        def s_exp_stage(hp, kt):
                            j0 = kt * P
                            o = max(0, j0 - q0)
                            s_pair = psS.tile([P, 1024], F32, tag="psS",
                                              name="s_pair")
                            for hs in range(2):
                                r0 = hs * HD
                                nc.tensor.matmul(
                                    s_pair[:, 512 * hs + o : 512 * (hs + 1)],
                                    kT[hp][r0 : r0 + HD, j0 : j0 + P],
                                    qT[hp][r0 : r0 + HD, q0 + o : q0 + 512],
                                    start=True,
                                    stop=True,
                                    tile_position=(r0, 0),
                                )
                            pt = pPp.tile([P, 1024], BF16, tag="pP",
                                          name="pt")
                            if o == 0:
                                # full tile: one 2D exp over both heads
                                nc.scalar.activation(pt[:], s_pair[:], AF.Exp)
                            else:
                                s3 = s_pair[:].rearrange(
                                    "p (h e) -> p h e", e=512
                                )
                                p3 = pt[:].rearrange("p (h e) -> p h e", e=512)
                                nc.scalar.activation(
                                    p3[:, :, o:512], s3[:, :, o:512], AF.Exp
                                )
                            if j0 >= q0:
                                p3 = pt[:].rearrange("p (h e) -> p h e", e=512)
                                tri3 = tri[:, None, :].broadcast_to([P, 2, P])
                                nc.vector.tensor_mul(
                                    p3[:, :, o : o + P], p3[:, :, o : o + P],
                                    tri3,
                                )
                            return pt

                        def y_stage(pidx, hp, kt, pt, first, final):
                            j0 = kt * P
                            o = max(0, j0 - q0)
                            for hs in range(2):
                                h = 2 * hp + hs
                                nc.tensor.matmul(
                                    y_ps[(pidx, hs)][:, o:512],
                                    vt[:, kt * HL * VW + h * VW
                                       : (kt * HL * VW + h * VW) + VW],
                                    pt[:, 512 * hs + o : 512 * (hs + 1)],
                                    start=first,
                                    stop=final,
                                )

                        # two pairs interleaved, depth-1 pipeline.  Full kt
                        # tiles first (their 'start' matmul clears the whole
                        # PSUM bank; diagonal tiles only touch cols o:512),
                        # diagonal tiles last -- their small exp lets ACT
                        # catch up right before the duo boundary.
                        kts = list(range(qc * 4)) + list(range(qc * 4, n_kt))
                        pts_prev = None
                        for i in range(n_kt + 1):
                            # drain deferred work first: a previous duo's
                            # norm_muls must be ISSUED before this duo's AV
                            # matmuls overwrite the recycled psY buffers,
                            # or the WAR dependency is never recorded
                            if deferred:
                                deferred.pop(0)()
                            pts_new = None
                            if i < n_kt:
                                pts_new = [
                                    s_exp_stage(hps[p], kts[i])
                                    for p in range(2)
                                ]
                            if pts_prev is not None:
                                for p in range(2):
                                    y_stage(p, hps[p], kts[i - 1],
                                            pts_prev[p], first=(i == 1),
                                            final=(i == n_kt))
                            pts_prev = pts_new

                        # normalization: reciprocal of the denominator row
                        # (PSUM partition 0) + partition-broadcast now; the
                        # multiplies go into the next duo's stage slots
                        # fast PSUM drain first: psY buffers free as soon as
                        # these copies land, so the next duo's AV matmuls
                        # are not gated on the reciprocal chain below
                        y_sbs = {}
                        for p in range(2):
                            for hs in range(2):
                                y_sb = ysbp.tile([VW, 512], F32, tag="ysb",
                                                 name="y_sb")
                                nc.vector.tensor_copy(
                                    y_sb[:], y_ps[(p, hs)][:]
                                )
                                y_sbs[(p, hs)] = y_sb
                        bcs = {}
                        for p in range(2):
                            for hs in range(2):
                                # exact DVE reciprocal is 8 cyc/elem/lane:
                                # 3.3us on a single-partition [1,512] row.
                                # Scatter the row across 128 partitions via
                                # DMA so it runs 128-wide (~0.3us), then
                                # gather back to partition 0 (which is also
                                # what partition_broadcast's ucode reads).
                                den_t = dentp.tile([P, 4], F32, tag="dent",
                                                   name="den_t")
                                nc.gpsimd.dma_start(
                                    den_t[:], y_sbs[(p, hs)][HD : HD + 1, :]
                                )
                                rct = dentp.tile([P, 4], F32, tag="rct",
                                                 name="rct")
                                nc.vector.reciprocal(rct[:], den_t[:])
                                rc = rcp.tile([1, 512], F32, tag="rc",
                                              name="rc")
                                nc.gpsimd.dma_start(rc[:], rct[:])
                                bc = bcsp.tile([HD, 512], F32, tag="bcs",
                                               name="bc")
                                nc.gpsimd.partition_broadcast(
                                    bc[:], rc[0:1, :]
                                )
                                bcs[(p, hs)] = bc

                        def norm_muls(y_sbs=y_sbs, bcs=bcs, hps=hps, q0=q0):
                            for p in range(2):
                                for hs in range(2):
                                    nc.vector.tensor_mul(
                                        yT[hps[p]][hs * HD : (hs + 1) * HD,
                                                   q0 : q0 + 512],
                                        y_sbs[(p, hs)][0:HD, :],
                                        bcs[(p, hs)][:],
                                    )

                        deferred.append(norm_muls)
                    for m in range(NCT):
                        deferred.append(
                            lambda qc=qc, m=m: emit_d_group(qc, m)
                        )
                while deferred:
                    deferred.pop(0)()
            _yTp_cm.__exit__(None, None, None)
    nc.compile()
    return nc


def make_in_maps(x, w_attn, b_attn, w_proj):
    scale = 1.0 / math.sqrt(HD)
    in_maps = []
    for core in range(N_CORES):
        b = core // 2
        g = core % 2
        h0 = g * HL
        rows = np.arange(h0 * HD, (h0 + HL) * HD)
        w_q = w_attn[rows, :] * scale           # fold 1/sqrt(hd) into Q
        w_k = w_attn[C + rows, :]
        w_v = w_attn[2 * C + rows, :]
        w_qkv_t = np.ascontiguousarray(
            np.concatenate([w_q, w_k, w_v], axis=0).T
        ).astype(NPBF16)
        w_proj_t = np.ascontiguousarray(w_proj[:, rows].T).astype(NPBF16)
        in_maps.append(
            {
                "x": np.ascontiguousarray(x[b]).astype(NPBF16),
                "w_qkv_t": w_qkv_t,
                "w_proj_t": w_proj_t,
            }
        )
    return in_maps


def _run(in_maps, trace=False, **kw):
    global _CACHED_NC
    if _CACHED_NC is None:
        _CACHED_NC = build_nc()
    return run_bass_kernel_spmd(
        _CACHED_NC, in_maps, core_ids=list(range(N_CORES)), trace=trace, **kw
    )


def kernel(x, w_attn, b_attn, w_proj, b_proj):
    x = np.asarray(x, dtype=np.float32)
    w_attn = np.asarray(w_attn, dtype=np.float32)
    b_attn = np.asarray(b_attn, dtype=np.float32)
    w_proj = np.asarray(w_proj, dtype=np.float32)
    b_proj = np.asarray(b_proj, dtype=np.float32)
    assert not np.any(b_attn), "kernel assumes b_attn == 0 (as in setup_inputs)"
    res = _run(make_in_maps(x, w_attn, b_attn, w_proj))
    out = np.empty((B, T, C), dtype=np.float32)
    for b in range(B):
        p0 = res.results[2 * b]["out_t"]
        p1 = res.results[2 * b + 1]["out_t"]
        out[b] = (p0 + p1).T + b_proj
    return out
